# Initial kernel scaffold
#
"""CRF (Dense + Viterbi decode) TRN2 Bass kernel.

Sharding: data-parallel over batch across 8 cores (4 rows each).
Per core:
  Phase 1: potentials = x @ W + b (+ boundary energies)  [PE matmul, fp32]
  Phase 2: Viterbi forward DP, storing v_t (state values) per step.
     scoresT[i,j] = A[i,j] + v[i]   (DVE tensor_scalar_add, fp32 2x mode)
     scores[j,i]  = PE block transposes -> PSUM
     v'[j] = max_i scores + pot_t[j] (DVE tensor_reduce(max) + tensor_tensor add)
  Host: traceback over v-history (exact fp32, matches reference op-for-op).
"""
import os
import numpy as np
from contextlib import ExitStack

import concourse.bass as bass
import concourse.tile as tile
from concourse import mybir, bacc
from concourse.bass_utils import run_bass_kernel_spmd
from concourse.bass import ts, ds

f32 = mybir.dt.float32

B, T, F, U = 32, 2048, 1024, 512
NCORES = 8
BPC = B // NCORES          # batch rows per core = 4
TOK = BPC * T              # tokens per core = 8192
NC_U = U // 128            # 4 u-chunks
NC_F = F // 128            # 8 f-chunks

_CACHE = {}


def build_nc(t_steps=T):
    nc = bacc.Bacc("TRN2", target_bir_lowering=False, debug=False, num_devices=NCORES)
    xT_ext = nc.declare_dram_parameter("xT", [F, TOK], f32, isOutput=False)
    W_ext = nc.declare_dram_parameter("Wsb", [128, NC_F * U], f32, isOutput=False)
    A_ext = nc.declare_dram_parameter("Asb", [128, NC_U * U], f32, isOutput=False)
    b_ext = nc.declare_dram_parameter("bcol", [128, NC_U], f32, isOutput=False)
    lb_ext = nc.declare_dram_parameter("lbcol", [128, NC_U], f32, isOutput=False)
    rb_ext = nc.declare_dram_parameter("rbcol", [128, NC_U], f32, isOutput=False)
    id_ext = nc.declare_dram_parameter("Id", [128, 128], f32, isOutput=False)
    pot_out = nc.declare_dram_parameter("pot", [BPC, T, U], f32, isOutput=True)
    vh_out = nc.declare_dram_parameter("vhist", [t_steps, BPC, U], f32, isOutput=True)

    with tile.TileContext(nc) as tc, ExitStack() as ctx:
        singles = ctx.enter_context(tc.tile_pool(name="singles", bufs=1))
        # pot resident in SBUF: [p, b, c, t] = 128 x 4 x 4 x T
        pot_sb = singles.tile([128, BPC, NC_U, T], f32, tag="pot")
        A_sb = singles.tile([128, NC_U, U], f32, tag="Asb")
        nc.sync.dma_start(out=A_sb[:], in_=A_ext[:].rearrange("p (c u) -> p c u", c=NC_U))
        ident = singles.tile([128, 128], f32, tag="id")
        nc.sync.dma_start(out=ident[:], in_=id_ext[:])
        bcol = singles.tile([128, NC_U], f32, tag="bc")
        nc.sync.dma_start(out=bcol[:], in_=b_ext[:])
        lbcol = singles.tile([128, NC_U], f32, tag="lb")
        nc.sync.dma_start(out=lbcol[:], in_=lb_ext[:])
        rbcol = singles.tile([128, NC_U], f32, tag="rb")
        nc.sync.dma_start(out=rbcol[:], in_=rb_ext[:])

        # ---------------- Phase 1: potentials ----------------
        with ExitStack() as p1:
            wpool = p1.enter_context(tc.tile_pool(name="wpool", bufs=1))
            xpool = p1.enter_context(tc.tile_pool(name="xpool", bufs=6))
            ps1 = p1.enter_context(tc.tile_pool(name="ps1", bufs=4, space="PSUM"))
            W_sb = wpool.tile([128, NC_F * U], f32, tag="W")
            nc.sync.dma_start(out=W_sb[:], in_=W_ext[:])
            ntt = TOK // 512  # 16 token tiles of 512
            for tt in range(ntt):
                b_idx, t0 = tt // (T // 512), (tt % (T // 512)) * 512
                xts = []
                for fc in range(NC_F):
                    xt = xpool.tile([128, 512], f32, tag="xt")
                    nc.sync.dma_start(out=xt[:], in_=xT_ext[fc * 128:(fc + 1) * 128, tt * 512:(tt + 1) * 512])
                    xts.append(xt)
                for uc in range(NC_U):
                    ps = ps1.tile([128, 512], f32, tag="p1")
                    for fc in range(NC_F):
                        nc.tensor.matmul(ps[:], W_sb[:, fc * U + uc * 128: fc * U + (uc + 1) * 128],
                                         xts[fc][:], start=(fc == 0), stop=(fc == NC_F - 1))
                    # evict + add bias b (per-partition scalar)
                    nc.vector.tensor_scalar_add(pot_sb[:, b_idx, uc, t0:t0 + 512], ps[:], bcol[:, uc:uc + 1])
        # boundary energies, in place on pot_sb
        for uc in range(NC_U):
            nc.vector.tensor_scalar_add(pot_sb[:, :, uc, 0:1], pot_sb[:, :, uc, 0:1], lbcol[:, uc:uc + 1])
            nc.vector.tensor_scalar_add(pot_sb[:, :, uc, T - 1:T], pot_sb[:, :, uc, T - 1:T], rbcol[:, uc:uc + 1])
        # stream potentials out (overlaps phase 2)
        for b_idx in range(BPC):
            for uc in range(NC_U):
                nc.sync.dma_start(
                    out=pot_out[b_idx, :, uc * 128:(uc + 1) * 128],
                    in_=pot_sb[:, b_idx, uc, :].rearrange("p t -> t p"))

        # ---------------- Phase 2: Viterbi forward ----------------
        scpool = ctx.enter_context(tc.tile_pool(name="scpool", bufs=2))
        pspool = ctx.enter_context(tc.tile_pool(name="pspool", bufs=2, space="PSUM"))
        vpool = ctx.enter_context(tc.tile_pool(name="vpool", bufs=1))
        mpool = ctx.enter_context(tc.tile_pool(name="mpool", bufs=3))

        v_ping = vpool.tile([128, BPC * NC_U], f32, tag="vping")
        v_pong = vpool.tile([128, BPC * NC_U], f32, tag="vpong")
        # v0 = pot[:, :, :, 0]
        nc.vector.tensor_copy(v_ping[:], pot_sb[:, :, :, 0].rearrange("p b c -> p (b c)"))
        nc.sync.dma_start(
            out=vh_out[0, :, :].rearrange("b (c p) -> p (b c)", p=128),
            in_=v_ping[:])

        def step(t, v_in, v_out):
            maxv = mpool.tile([128, BPC * NC_U], f32, tag="maxv")
            for b_idx in range(BPC):
                sc = scpool.tile([128, NC_U, U], f32, tag="sc")
                for ci in range(NC_U):
                    nc.vector.tensor_scalar_add(sc[:, ci, :], A_sb[:, ci, :],
                                                v_in[:, b_idx * NC_U + ci: b_idx * NC_U + ci + 1])
                psb = pspool.tile([128, NC_U, 512], f32, tag="ps")
                for cj in range(NC_U):
                    for ci in range(NC_U):
                        nc.tensor.transpose(psb[:, cj, ts(ci, 128)], sc[:, ci, ts(cj, 128)], ident[:])
                nc.vector.tensor_reduce(
                    maxv[:, b_idx * NC_U:(b_idx + 1) * NC_U], psb[:],
                    axis=mybir.AxisListType.X, op=mybir.AluOpType.max)
            nc.vector.tensor_tensor(
                v_out[:], maxv[:],
                pot_sb[:, :, :, ds(t, 1)].rearrange("p b c o -> p (b c o)"),
                op=mybir.AluOpType.add)
            nc.sync.dma_start(
                out=vh_out[ds(t, 1), :, :].rearrange("o b (c p) -> p (o b c)", p=128),
                in_=v_out[:])

        UNROLL = 16

        def body(iv):
            for k in range(UNROLL):
                vin, vout = (v_ping, v_pong) if k % 2 == 0 else (v_pong, v_ping)
                step(iv + k, vin, vout)

        n_bulk = ((t_steps - 1) // UNROLL) * UNROLL
        if n_bulk > 0:
            tc.For_i_unrolled_general(1, 1 + n_bulk, 1,
                                      lambda iv, unroll: [step(iv + i,
                                                               *( (v_ping, v_pong) if i % 2 == 0 else (v_pong, v_ping)))
                                                          for i in range(unroll)],
                                      max_unroll=UNROLL)
        for k in range(n_bulk + 1, t_steps):
            vin, vout = (v_ping, v_pong) if (k - 1) % 2 == 0 else (v_pong, v_ping)
            step(k, vin, vout)
    nc.compile()
    return nc


def kernel(x, W, b, transitions, left_boundary, right_boundary):
    x = np.ascontiguousarray(np.asarray(x, np.float32))
    W = np.asarray(W, np.float32)
    b = np.asarray(b, np.float32)
    A = np.asarray(transitions, np.float32)
    lb = np.asarray(left_boundary, np.float32)
    rb = np.asarray(right_boundary, np.float32)

    t_steps = int(os.environ.get("CRF_T_STEPS", T))
    if "nc" not in _CACHE or _CACHE.get("t") != t_steps:
        _CACHE["nc"] = build_nc(t_steps)
        _CACHE["t"] = t_steps
    nc = _CACHE["nc"]

    W_sb = W.reshape(NC_F, 128, NC_U, 128).transpose(1, 0, 2, 3).reshape(128, NC_F * U)
    A_sb = A.reshape(NC_U, 128, U).transpose(1, 0, 2).reshape(128, NC_U * U)
    bcol = b.reshape(NC_U, 128).T.copy()
    lbcol = lb.reshape(NC_U, 128).T.copy()
    rbcol = rb.reshape(NC_U, 128).T.copy()
    ident = np.eye(128, dtype=np.float32)

    in_maps = []
    for c in range(NCORES):
        xs = x[c * BPC:(c + 1) * BPC]  # [4, T, F]
        xTs = np.ascontiguousarray(xs.reshape(BPC * T, F).T)  # [F, 8192]
        in_maps.append({
            "xT": xTs, "Wsb": W_sb, "Asb": A_sb, "bcol": bcol,
            "lbcol": lbcol, "rbcol": rbcol, "Id": ident,
        })

    res = run_bass_kernel_spmd(nc, in_maps, list(range(NCORES)), trace=False)

    pot = np.concatenate([res.results[c]["pot"] for c in range(NCORES)], axis=0)
    vhist = np.stack([res.results[c]["vhist"] for c in range(NCORES)], axis=0)  # [8, t, 4, U]

    # host traceback (exact fp32, identical op order to reference)
    vhist = vhist.transpose(1, 0, 2, 3).reshape(t_steps, B, U)
    tags = np.zeros((B, t_steps), np.int32)
    cur = vhist[t_steps - 1].argmax(axis=1).astype(np.int32)
    tags[:, t_steps - 1] = cur
    AT = np.ascontiguousarray(A.T)  # AT[j, i] = A[i, j]
    rows = np.arange(B)
    for t in range(t_steps - 1, 0, -1):
        scores = vhist[t - 1] + AT[cur]  # [B, U] fp32
        cur = scores.argmax(axis=1).astype(np.int32)
        tags[:, t - 1] = cur

    seq_len = np.full((B,), T, dtype=np.int32)
    return tags, pot, seq_len, np.asarray(transitions)


# revision 2
# speedup vs baseline: 1.4995x; 1.4995x over previous
"""CRF (Dense + Viterbi decode) TRN2 Bass kernel.

Sharding: data-parallel over batch across 8 cores (4 rows each).
Per core:
  Phase 1: potentials = x @ W + b (+ boundary energies)  [PE matmul, fp32]
  Phase 2: Viterbi forward DP, storing v_t (state values) per step.
     scoresT[i,j] = A[i,j] + v[i]   (DVE tensor_scalar_add, fp32 2x mode)
     scores[j,i]  = PE block transposes -> PSUM
     v'[j] = max_i scores + pot_t[j] (DVE tensor_reduce(max) + tensor_tensor add)
  Host: traceback over v-history (exact fp32, matches reference op-for-op).
"""
import os
import numpy as np
from contextlib import ExitStack

import concourse.bass as bass
import concourse.tile as tile
from concourse import mybir, bacc
from concourse.bass_utils import run_bass_kernel_spmd
from concourse.bass import ts, ds

f32 = mybir.dt.float32

B, T, F, U = 32, 2048, 1024, 512
NCORES = 8
BPC = B // NCORES          # batch rows per core = 4
TOK = BPC * T              # tokens per core = 8192
NC_U = U // 128            # 4 u-chunks
NC_F = F // 128            # 8 f-chunks

_CACHE = {}


def build_nc(t_steps=T):
    nc = bacc.Bacc("TRN2", target_bir_lowering=False, debug=False, num_devices=NCORES)
    xT_ext = nc.declare_dram_parameter("xT", [F, TOK], f32, isOutput=False)
    W_ext = nc.declare_dram_parameter("Wsb", [128, NC_F * U], f32, isOutput=False)
    A_ext = nc.declare_dram_parameter("Asb", [128, NC_U * U], f32, isOutput=False)
    b_ext = nc.declare_dram_parameter("bcol", [128, NC_U], f32, isOutput=False)
    lb_ext = nc.declare_dram_parameter("lbcol", [128, NC_U], f32, isOutput=False)
    rb_ext = nc.declare_dram_parameter("rbcol", [128, NC_U], f32, isOutput=False)
    id_ext = nc.declare_dram_parameter("Id", [128, 128], f32, isOutput=False)
    pot_out = nc.declare_dram_parameter("pot", [BPC, T, U], f32, isOutput=True)
    vh_out = nc.declare_dram_parameter("vhist", [t_steps, BPC, U], f32, isOutput=True)

    with tile.TileContext(nc) as tc, ExitStack() as ctx:
        singles = ctx.enter_context(tc.tile_pool(name="singles", bufs=1))
        # pot resident in SBUF: [p, b, c, t] = 128 x 4 x 4 x T
        pot_sb = singles.tile([128, BPC, NC_U, T], f32, tag="pot")
        A_sb = singles.tile([128, NC_U, U], f32, tag="Asb")
        nc.sync.dma_start(out=A_sb[:], in_=A_ext[:].rearrange("p (c u) -> p c u", c=NC_U))
        ident = singles.tile([128, 128], f32, tag="id")
        nc.sync.dma_start(out=ident[:], in_=id_ext[:])
        bcol = singles.tile([128, NC_U], f32, tag="bc")
        nc.sync.dma_start(out=bcol[:], in_=b_ext[:])
        lbcol = singles.tile([128, NC_U], f32, tag="lb")
        nc.sync.dma_start(out=lbcol[:], in_=lb_ext[:])
        rbcol = singles.tile([128, NC_U], f32, tag="rb")
        nc.sync.dma_start(out=rbcol[:], in_=rb_ext[:])

        # ---------------- Phase 1: potentials ----------------
        with ExitStack() as p1:
            wpool = p1.enter_context(tc.tile_pool(name="wpool", bufs=1))
            xpool = p1.enter_context(tc.tile_pool(name="xpool", bufs=6))
            ps1 = p1.enter_context(tc.tile_pool(name="ps1", bufs=4, space="PSUM"))
            W_sb = wpool.tile([128, NC_F * U], f32, tag="W")
            nc.sync.dma_start(out=W_sb[:], in_=W_ext[:])
            ntt = TOK // 512  # 16 token tiles of 512
            for tt in range(ntt):
                b_idx, t0 = tt // (T // 512), (tt % (T // 512)) * 512
                xts = []
                for fc in range(NC_F):
                    xt = xpool.tile([128, 512], f32, tag="xt")
                    nc.sync.dma_start(out=xt[:], in_=xT_ext[fc * 128:(fc + 1) * 128, tt * 512:(tt + 1) * 512])
                    xts.append(xt)
                for uc in range(NC_U):
                    ps = ps1.tile([128, 512], f32, tag="p1")
                    for fc in range(NC_F):
                        nc.tensor.matmul(ps[:], W_sb[:, fc * U + uc * 128: fc * U + (uc + 1) * 128],
                                         xts[fc][:], start=(fc == 0), stop=(fc == NC_F - 1))
                    # evict + add bias b (per-partition scalar)
                    nc.vector.tensor_scalar_add(pot_sb[:, b_idx, uc, t0:t0 + 512], ps[:], bcol[:, uc:uc + 1])
        # boundary energies, in place on pot_sb
        for uc in range(NC_U):
            nc.vector.tensor_scalar_add(pot_sb[:, :, uc, 0:1], pot_sb[:, :, uc, 0:1], lbcol[:, uc:uc + 1])
            nc.vector.tensor_scalar_add(pot_sb[:, :, uc, T - 1:T], pot_sb[:, :, uc, T - 1:T], rbcol[:, uc:uc + 1])
        # stream potentials out (overlaps phase 2)
        for b_idx in range(BPC):
            for uc in range(NC_U):
                nc.sync.dma_start(
                    out=pot_out[b_idx, :, uc * 128:(uc + 1) * 128].rearrange("t p -> p t"),
                    in_=pot_sb[:, b_idx, uc, :])

        # ---------------- Phase 2: Viterbi forward ----------------
        scpool = ctx.enter_context(tc.tile_pool(name="scpool", bufs=2))
        pspool = ctx.enter_context(tc.tile_pool(name="pspool", bufs=2, space="PSUM"))
        vpool = ctx.enter_context(tc.tile_pool(name="vpool", bufs=1))
        mpool = ctx.enter_context(tc.tile_pool(name="mpool", bufs=3))

        v_ping = vpool.tile([128, BPC * NC_U], f32, tag="vping")
        v_pong = vpool.tile([128, BPC * NC_U], f32, tag="vpong")
        # v0 = pot[:, :, :, 0]
        nc.vector.tensor_copy(v_ping[:], pot_sb[:, :, :, 0].rearrange("p b c -> p (b c)"))
        nc.sync.dma_start(
            out=vh_out[0, :, :].rearrange("b (c p) -> p (b c)", p=128),
            in_=v_ping[:])

        def step(t, v_in, v_out):
            maxv = mpool.tile([128, BPC * NC_U], f32, tag="maxv")
            for b_idx in range(BPC):
                sc = scpool.tile([128, NC_U, U], f32, tag="sc")
                for ci in range(NC_U):
                    nc.vector.tensor_scalar_add(sc[:, ci, :], A_sb[:, ci, :],
                                                v_in[:, b_idx * NC_U + ci: b_idx * NC_U + ci + 1])
                psb = pspool.tile([128, NC_U, 512], f32, tag="ps")
                for cj in range(NC_U):
                    for ci in range(NC_U):
                        nc.tensor.transpose(psb[:, cj, ts(ci, 128)], sc[:, ci, ts(cj, 128)], ident[:])
                nc.vector.tensor_reduce(
                    maxv[:, b_idx * NC_U:(b_idx + 1) * NC_U], psb[:],
                    axis=mybir.AxisListType.X, op=mybir.AluOpType.max)
            nc.vector.tensor_tensor(
                v_out[:], maxv[:],
                pot_sb[:, :, :, ds(t, 1)].rearrange("p b c o -> p (b c o)"),
                op=mybir.AluOpType.add)
            nc.sync.dma_start(
                out=vh_out[ds(t, 1), :, :].rearrange("o b (c p) -> p (o b c)", p=128),
                in_=v_out[:])

        UNROLL = 16

        def body(iv):
            for k in range(UNROLL):
                vin, vout = (v_ping, v_pong) if k % 2 == 0 else (v_pong, v_ping)
                step(iv + k, vin, vout)

        n_bulk = ((t_steps - 1) // UNROLL) * UNROLL
        if n_bulk > 0:
            tc.For_i_unrolled_general(1, 1 + n_bulk, 1,
                                      lambda iv, unroll: [step(iv + i,
                                                               *( (v_ping, v_pong) if i % 2 == 0 else (v_pong, v_ping)))
                                                          for i in range(unroll)],
                                      max_unroll=UNROLL)
        for k in range(n_bulk + 1, t_steps):
            vin, vout = (v_ping, v_pong) if (k - 1) % 2 == 0 else (v_pong, v_ping)
            step(k, vin, vout)
    nc.compile()
    return nc


def kernel(x, W, b, transitions, left_boundary, right_boundary):
    x = np.ascontiguousarray(np.asarray(x, np.float32))
    W = np.asarray(W, np.float32)
    b = np.asarray(b, np.float32)
    A = np.asarray(transitions, np.float32)
    lb = np.asarray(left_boundary, np.float32)
    rb = np.asarray(right_boundary, np.float32)

    t_steps = int(os.environ.get("CRF_T_STEPS", T))
    if "nc" not in _CACHE or _CACHE.get("t") != t_steps:
        _CACHE["nc"] = build_nc(t_steps)
        _CACHE["t"] = t_steps
    nc = _CACHE["nc"]

    W_sb = W.reshape(NC_F, 128, NC_U, 128).transpose(1, 0, 2, 3).reshape(128, NC_F * U)
    A_sb = A.reshape(NC_U, 128, U).transpose(1, 0, 2).reshape(128, NC_U * U)
    bcol = b.reshape(NC_U, 128).T.copy()
    lbcol = lb.reshape(NC_U, 128).T.copy()
    rbcol = rb.reshape(NC_U, 128).T.copy()
    ident = np.eye(128, dtype=np.float32)

    in_maps = []
    for c in range(NCORES):
        xs = x[c * BPC:(c + 1) * BPC]  # [4, T, F]
        xTs = np.ascontiguousarray(xs.reshape(BPC * T, F).T)  # [F, 8192]
        in_maps.append({
            "xT": xTs, "Wsb": W_sb, "Asb": A_sb, "bcol": bcol,
            "lbcol": lbcol, "rbcol": rbcol, "Id": ident,
        })

    res = run_bass_kernel_spmd(nc, in_maps, list(range(NCORES)), trace=False)

    pot = np.concatenate([res.results[c]["pot"] for c in range(NCORES)], axis=0)
    vhist = np.stack([res.results[c]["vhist"] for c in range(NCORES)], axis=0)  # [8, t, 4, U]

    # host traceback (exact fp32, identical op order to reference)
    vhist = vhist.transpose(1, 0, 2, 3).reshape(t_steps, B, U)
    tags = np.zeros((B, t_steps), np.int32)
    cur = vhist[t_steps - 1].argmax(axis=1).astype(np.int32)
    tags[:, t_steps - 1] = cur
    AT = np.ascontiguousarray(A.T)  # AT[j, i] = A[i, j]
    rows = np.arange(B)
    for t in range(t_steps - 1, 0, -1):
        scores = vhist[t - 1] + AT[cur]  # [B, U] fp32
        cur = scores.argmax(axis=1).astype(np.int32)
        tags[:, t - 1] = cur

    seq_len = np.full((B,), T, dtype=np.int32)
    return tags, pot, seq_len, np.asarray(transitions)
